# revision 1
# baseline (speedup 1.0000x reference)
"""TRN2 Bass kernel for the 4-layer encoder-with-reaches model
(nn_EncoderPreTre: B=8, S=512, D=1024, H=16 heads, NL=4 layers).

kernel(**inputs) takes the FULL inputs (src, reaches, emb_table,
qw/kw/vw/ow) and returns the full output tuple (emb, x) matching
reference.reference(). Distribution: data-parallel over the batch —
core b computes batch element b end to end (B == 8 == n_cores); the
embedding-row gather and per-batch contrib vectors are the host-side
sharding step.

Numerics: this model amplifies matmul rounding ~1000x (logits reach
5.6e6; contrib scaling grows x ~40x per layer), so bf16/tf32-class
matmuls fail. Everything runs at fp32 fidelity:
  - scores / P-transposes / attention-value / output projections are
    native fp32 matmuls (4 cycles/row on the PE);
  - the q/k/v/OV projections use a 3-term float32r hi/lo split
    (Wh@xh + Wh@xl + Wl@xh, each 1 cycle/row) with weights pre-split on
    the host into tf32-representable halves and the residual stream kept
    as a float32r (hi, lo) pair — fp32-equivalent accuracy at ~2.6x the
    fp32 matmul throughput.

Per-core dataflow (residual transposed: x^T [1024, 512] in SBUF):
  P1: q^T = (qw/8)^T-proj, k^T, v in [s,do] layout with v' = v*reaches
      fused into the PSUM->SBUF copy.
  P2 per head: scores[q,k] -> row-max (DVE reduce, negated) ->
      E = exp(s-m) with row-sum Z from the same ACT op (accum_out; the
      softmax numerator/denominator use the same PSUM values, keeping it
      consistent at huge logit scale) -> P = (E*(-c/Z)[q])*diagmask in
      one scalar_tensor_tensor -> P transposed 128x128-blockwise via PE
      transpose-mode -> M2T[dk,q] = sum_k v'[k,dk]*PT[k,q].
  P3: x += (OV@x)*c + ow-proj(concatT), OV = ow@vw folded on the host so
      the reference's "v - P@v'" becomes two accumulating projections.

Implementation notes:
  - Residual stored as f32r pair (xhi, xlo): x ~= xhi + xlo to ~2^-22.
  - Weights for q/k/v/OV pre-split on host into tf32-representable hi/lo
    parts, DMA'd as float32r tiles (verifier accepts f32r-typed producers).
  - Projections computed as 3-term splits: Wh@xh + Wh@xl + Wl@xh, each a
    1-cycle/row f32r matmul (vs 4 cycles/row for fp32).
  - scores / transposes / M2 / OW2 remain fp32 (their operands are
    device-produced fp32 tensors; splitting them costs more DVE than it
    saves PE).
"""
import numpy as np

import concourse.tile as tile
from concourse import bacc, mybir
from concourse.bass_utils import run_bass_kernel_spmd

F32 = mybir.dt.float32
F32R = mybir.dt.float32r
BF16 = mybir.dt.bfloat16
AX = mybir.AxisListType
OP = mybir.AluOpType
AF = mybir.ActivationFunctionType

B, S, D, H, DK, NL = 8, 512, 1024, 16, 64, 4
QC = S // 128
KC = S // 128
DC = D // 128


TRACE = False        # test harness sets True for neuron-profile capture
LAST_RESULT = None   # BassKernelResults of the last kernel() call
_NC_CACHE = {}


def _build(n_layers=NL, n_cores=8):
    nc = bacc.Bacc("TRN2", target_bir_lowering=False, debug=False,
                   num_devices=n_cores)
    d_x0 = nc.dram_tensor("x0t", [D, S], F32, kind="ExternalInput").ap()
    dw = {}
    for nm in ["wq", "wk", "wv", "wov"]:
        dw[nm + "h"] = nc.dram_tensor(nm + "h", [NL, D, D], F32R,
                                      kind="ExternalInput").ap()
        dw[nm + "l"] = nc.dram_tensor(nm + "l", [NL, D, D], F32R,
                                      kind="ExternalInput").ap()
    dw["wo"] = nc.dram_tensor("wo", [NL, D, D], F32, kind="ExternalInput").ap()
    d_cb = nc.dram_tensor("cb", [128, S], F32, kind="ExternalInput").ap()
    d_negc = nc.dram_tensor("negc", [128, QC], F32, kind="ExternalInput").ap()
    d_rr = nc.dram_tensor("rr", [128, KC], F32, kind="ExternalInput").ap()
    d_mask = nc.dram_tensor("maskq", [QC, 128, S], mybir.dt.bfloat16, kind="ExternalInput").ap()
    d_id = nc.dram_tensor("ident", [128, 128], F32, kind="ExternalInput").ap()
    d_out = nc.dram_tensor("xt", [D, S], F32, kind="ExternalOutput").ap()

    with tile.TileContext(nc) as tc:
        _emit(nc, tc, n_layers, d_x0, dw,
              d_cb, d_negc, d_rr, d_mask, d_id, d_out)
    nc.compile()
    return nc


def _emit(nc, tc, n_layers, d_x0, dw, d_cb, d_negc, d_rr, d_mask, d_id, d_out):
    ctx_pools = []

    def pool(name, bufs, space="SBUF"):
        p = tc.tile_pool(name=name, bufs=bufs, space=space)
        ctx_pools.append(p)
        return p.__enter__()

    const = pool("const", 1)
    xpool = pool("x", 1)
    actp = pool("act", 1)
    wpool = pool("w", 1)          # 16 tags (hi/lo per ki); ki-granular overlap
    epool = pool("E", 2)
    ppool = pool("P", 4)
    ptpool = pool("PT", 4)
    small = pool("small", 3)
    tmp8 = pool("tmp8", 1)
    psA = pool("psA", 3, "PSUM")
    psB = pool("psB", 2, "PSUM")
    psC = pool("psC", 2, "PSUM")

    cb = const.tile([128, S], F32)
    nc.sync.dma_start(cb[:], d_cb)
    negc = const.tile([128, QC], F32)
    nc.sync.dma_start(negc[:], d_negc)
    rr = const.tile([128, KC], F32)
    nc.sync.dma_start(rr[:], d_rr)
    ident = const.tile([128, 128], F32)
    nc.sync.dma_start(ident[:], d_id)
    masks = []
    for t in range(QC):
        mt = const.tile([128, S], BF16, tag=f"mask{t}", name=f"mask{t}")
        nc.sync.dma_start(mt[:], d_mask[t])
        masks.append(mt)

    # residual pair: x ~= xhi + xlo (f32r each)
    xhi, xlo = [], []
    for c in range(DC):
        xh = xpool.tile([128, S], F32R, tag=f"xh{c}", name=f"xh{c}")
        xl = xpool.tile([128, S], F32R, tag=f"xl{c}", name=f"xl{c}")
        xhi.append(xh)
        xlo.append(xl)
    for c in range(DC):
        xf = tmp8.tile([128, S], F32, tag="xn", name=f"x0f{c}", bufs=1)
        nc.sync.dma_start(xf[:], d_x0[c * 128:(c + 1) * 128, :])
        nc.vector.tensor_copy(xhi[c][:], xf[:])
        nc.vector.tensor_tensor(xlo[c][:], xf[:], xhi[c][:], op=OP.subtract)

    for l in range(n_layers):
        def load_split(nm):
            his, los = [], []
            for ki in range(DC):
                wh = wpool.tile([128, D], F32R, tag=f"w{ki}h", name=f"{nm}h{ki}_{l}")
                nc.sync.dma_start(wh[:], dw[nm + "h"][l, ki * 128:(ki + 1) * 128, :])
                wl = wpool.tile([128, D], F32R, tag=f"w{ki}l", name=f"{nm}l{ki}_{l}")
                nc.sync.dma_start(wl[:], dw[nm + "l"][l, ki * 128:(ki + 1) * 128, :])
                his.append(wh)
                los.append(wl)
            return his, los

        def proj_split(his, los, outtag, rhs_hi, rhs_lo):
            outs = []
            for c in range(DC):
                p = psA.tile([128, S], F32, tag="psA", name=f"pp{outtag}{c}_{l}")
                n_mm = 3 * DC
                i_mm = 0
                sl = slice(c * 128, (c + 1) * 128)
                for ki in range(DC):
                    for lhsT, rhs in ((his[ki][:, sl], rhs_hi[ki][:]),
                                      (his[ki][:, sl], rhs_lo[ki][:]),
                                      (los[ki][:, sl], rhs_hi[ki][:])):
                        nc.tensor.matmul(
                            p[:], lhsT, rhs, start=(i_mm == 0),
                            stop=(i_mm == n_mm - 1), skip_group_check=True)
                        i_mm += 1
                o = actp.tile([128, S], F32, tag=f"{outtag}{c}",
                              name=f"{outtag}{c}_{l}")
                nc.vector.tensor_copy(o[:], p[:])
                outs.append(o)
            return outs

        qh, ql = load_split("wq")
        qt = proj_split(qh, ql, "qt", xhi, xlo)
        kh, kl = load_split("wk")
        kt = proj_split(kh, kl, "kt", xhi, xlo)

        vh, vl = load_split("wv")
        vp = []
        for sc in range(KC):
            vtile = actp.tile([128, D], F32, tag=f"vp{sc}", name=f"vp{sc}_{l}")
            ssl = slice(sc * 128, (sc + 1) * 128)
            for half in range(2):
                hsl = slice(half * 512, (half + 1) * 512)
                p = psA.tile([128, S], F32, tag="psA", name=f"pv{sc}{half}_{l}")
                n_mm = 3 * DC
                i_mm = 0
                for ki in range(DC):
                    for lhsT, rhs in ((xhi[ki][:, ssl], vh[ki][:, hsl]),
                                      (xlo[ki][:, ssl], vh[ki][:, hsl]),
                                      (xhi[ki][:, ssl], vl[ki][:, hsl])):
                        nc.tensor.matmul(p[:], lhsT, rhs, start=(i_mm == 0),
                                         stop=(i_mm == n_mm - 1),
                                         skip_group_check=True)
                        i_mm += 1
                nc.vector.tensor_scalar(
                    vtile[:, hsl], p[:], rr[:, sc:sc + 1], None, op0=OP.mult)
            vp.append(vtile)

        concatT = [actp.tile([128, S], F32, tag=f"cc{c}", name=f"cc{c}_{l}")
                   for c in range(DC)]
        for h in range(H):
            hp = h // 2
            hb = (h % 2) * 64
            qsl = qt[hp][hb:hb + 64, :]
            ksl = kt[hp][hb:hb + 64, :]

            negm = small.tile([128, QC], F32, tag="negm", name=f"negm{h}_{l}")
            zst = small.tile([128, QC], F32, tag="zst", name=f"zst{h}_{l}")
            sc_t = small.tile([128, QC], F32, tag="scl", name=f"scl{h}_{l}")
            Ps = []
            for t in range(QC):
                ps = psA.tile([128, S], F32, tag="psA", name=f"sc{h}{t}_{l}")
                nc.tensor.matmul(ps[:], qsl[:, t * 128:(t + 1) * 128], ksl,
                                 start=True, stop=True)
                nc.vector.tensor_reduce(
                    negm[:, t:t + 1], ps[:], axis=AX.X, op=OP.max, negate=True)
                e = epool.tile([128, S], F32, tag="E", name=f"e{h}{t}_{l}")
                nc.scalar.activation(e[:], ps[:], AF.Exp,
                                     bias=negm[:, t:t + 1], scale=1.0,
                                     accum_out=zst[:, t:t + 1])
                nc.vector.reciprocal(sc_t[:, t:t + 1], zst[:, t:t + 1])
                nc.vector.tensor_tensor(
                    sc_t[:, t:t + 1], sc_t[:, t:t + 1], negc[:, t:t + 1],
                    op=OP.mult)
                p = ppool.tile([128, S], F32, tag="P", name=f"p{h}{t}_{l}")
                nc.vector.scalar_tensor_tensor(
                    p[:], e[:], sc_t[:, t:t + 1], masks[t][:],
                    op0=OP.mult, op1=OP.mult)
                Ps.append(p)

            PTs = []
            for kc in range(KC):
                tp = psB.tile([128, S], F32, tag="psB", name=f"tp{h}{kc}_{l}")
                for t in range(QC):
                    nc.tensor.matmul(
                        tp[:, t * 128:(t + 1) * 128],
                        Ps[t][:, kc * 128:(kc + 1) * 128], ident[:],
                        is_transpose=True, start=(t == 0), stop=(t == QC - 1),
                        skip_group_check=True)
                pt_sb = ptpool.tile([128, S], F32, tag="PT", name=f"pt{h}{kc}_{l}")
                if kc % 2 == 0:
                    nc.vector.tensor_copy(pt_sb[:], tp[:])
                else:
                    nc.scalar.copy(pt_sb[:], tp[:])
                PTs.append(pt_sb)

            m2 = psC.tile([128, S], F32, tag="psC", name=f"m2{h}_{l}")
            off = hb
            for kc in range(KC):
                nc.tensor.matmul(
                    m2[off:off + 64, :], vp[kc][:, h * 64:h * 64 + 64],
                    PTs[kc][:], start=(kc == 0), stop=(kc == KC - 1))
            nc.vector.tensor_copy(concatT[hp][hb:hb + 64, :], m2[off:off + 64, :])

        ovh, ovl = load_split("wov")
        wo_t = []
        for ki in range(DC):
            wt = wpool.tile([128, D], F32, tag=f"w{ki}h", name=f"wo{ki}_{l}")
            nc.sync.dma_start(wt[:], dw["wo"][l, ki * 128:(ki + 1) * 128, :])
            wo_t.append(wt)

        t1s = []
        for c in range(DC):
            pov = psC.tile([128, S], F32, tag="psC", name=f"pov{c}_{l}")
            n_mm = 3 * DC
            i_mm = 0
            sl = slice(c * 128, (c + 1) * 128)
            for ki in range(DC):
                for lhsT, rhs in ((ovh[ki][:, sl], xhi[ki][:]),
                                  (ovh[ki][:, sl], xlo[ki][:]),
                                  (ovl[ki][:, sl], xhi[ki][:])):
                    nc.tensor.matmul(pov[:], lhsT, rhs, start=(i_mm == 0),
                                     stop=(i_mm == n_mm - 1),
                                     skip_group_check=True)
                    i_mm += 1
            t1 = tmp8.tile([128, S], F32, tag=f"t1{c}", name=f"t1{c}_{l}", bufs=1)
            nc.vector.tensor_tensor(t1[:], pov[:], cb[:], op=OP.mult)
            t1s.append(t1)
        for c in range(DC):
            pow_ = psB.tile([128, S], F32, tag="psB", name=f"pow{c}_{l}")
            for ki in range(DC):
                nc.tensor.matmul(
                    pow_[:], wo_t[ki][:, c * 128:(c + 1) * 128], concatT[ki][:],
                    start=(ki == 0), stop=(ki == DC - 1))
            # rebuild full-precision x, add delta, re-split into the pair
            xt_new = tmp8.tile([128, S], F32, tag="xn", name=f"xn{c}_{l}", bufs=1)
            nc.vector.tensor_tensor(xt_new[:], xhi[c][:], xlo[c][:], op=OP.add)
            nc.vector.tensor_tensor(xt_new[:], xt_new[:], pow_[:], op=OP.add)
            nc.vector.tensor_tensor(xt_new[:], xt_new[:], t1s[c][:], op=OP.add)
            if l == n_layers - 1:
                nc.sync.dma_start(d_out[c * 128:(c + 1) * 128, :], xt_new[:])
            else:
                nc.vector.tensor_copy(xhi[c][:], xt_new[:])
                nc.vector.tensor_tensor(xlo[c][:], xt_new[:], xhi[c][:],
                                        op=OP.subtract)

    for p in reversed(ctx_pools):
        p.__exit__(None, None, None)


# ---------------- host side ----------------

def _t13(a):
    u = np.ascontiguousarray(np.asarray(a, np.float32)).view(np.uint32)
    r = (u + np.uint32(1 << 12)) & ~np.uint32((1 << 13) - 1)
    return r.view(np.float32)


def _host_prep(src, reaches, emb_table, qw, kw, vw, ow):
    src = np.asarray(src)
    reaches = np.asarray(reaches, dtype=np.float32)
    emb_table = np.asarray(emb_table, dtype=np.float32)
    emb = emb_table[src]
    rs = reaches.sum(-1, keepdims=True)
    contrib = ((rs - reaches) / (rs + 1e-9) * (1.0 - reaches) * 100.0
               ).astype(np.float32)

    qw = np.asarray(qw, np.float32); kw = np.asarray(kw, np.float32)
    vw = np.asarray(vw, np.float32); ow = np.asarray(ow, np.float32)
    wq = np.ascontiguousarray(np.transpose(qw, (0, 2, 1)) * 0.125)
    wk = np.ascontiguousarray(np.transpose(kw, (0, 2, 1)))
    wv = np.ascontiguousarray(np.transpose(vw, (0, 2, 1)))
    wo = np.ascontiguousarray(np.transpose(ow, (0, 2, 1)))
    wov = np.stack([
        np.ascontiguousarray(
            (ow[l].astype(np.float64) @ vw[l].astype(np.float64)).T
        ).astype(np.float32)
        for l in range(NL)])

    def split(w):
        h = _t13(w)
        lo = _t13(w - h)
        return np.ascontiguousarray(h), np.ascontiguousarray(lo)

    wqh, wql = split(wq)
    wkh, wkl = split(wk)
    wvh, wvl = split(wv)
    wovh, wovl = split(wov)

    import ml_dtypes
    maskq = np.ones((QC, 128, S), ml_dtypes.bfloat16)
    idx = np.arange(128)
    diagval = np.float32(1.0) - np.float32(0.999999)
    for t in range(QC):
        maskq[t, idx, t * 128 + idx] = ml_dtypes.bfloat16(diagval)
    ident = np.eye(128, dtype=np.float32)

    shared = dict(wqh=wqh, wql=wql, wkh=wkh, wkl=wkl, wvh=wvh, wvl=wvl,
                  wovh=wovh, wovl=wovl, wo=wo, maskq=maskq, ident=ident)
    in_maps = []
    for b in range(B):
        in_maps.append(dict(
            shared,
            x0t=np.ascontiguousarray(emb[b].T),
            cb=np.ascontiguousarray(
                np.broadcast_to(contrib[b][None, :], (128, S))),
            negc=np.ascontiguousarray(-contrib[b].reshape(QC, 128).T),
            rr=np.ascontiguousarray(reaches[b].reshape(KC, 128).T),
        ))
    return emb, in_maps


def kernel(src, reaches, emb_table, qw, kw, vw, ow):
    global LAST_RESULT
    if "nc" not in _NC_CACHE:
        _NC_CACHE["nc"] = _build(n_layers=NL, n_cores=B)
    nc = _NC_CACHE["nc"]
    emb, in_maps = _host_prep(src, reaches, emb_table, qw, kw, vw, ow)
    res = run_bass_kernel_spmd(nc, in_maps, core_ids=list(range(B)),
                               trace=TRACE)
    LAST_RESULT = res
    x = np.stack([r["xt"].T for r in res.results]).astype(np.float32)
    return emb, x



# revision 3
# speedup vs baseline: 1.3207x; 1.3207x over previous
"""TRN2 Bass kernel for the 4-layer encoder-with-reaches model
(nn_EncoderPreTre: B=8, S=512, D=1024, H=16 heads, NL=4 layers).

kernel(**inputs) takes the FULL inputs (src, reaches, emb_table,
qw/kw/vw/ow) and returns the full output tuple (emb, x) matching
reference.reference(). Distribution: data-parallel over the batch —
core b computes batch element b end to end (B == 8 == n_cores); the
embedding-row gather and per-batch contrib vectors are the host-side
sharding step.

Numerics: the model amplifies score-path rounding hugely (logits reach
~4e6 by layer 3; contrib scaling grows x ~40x per layer), so the
residual x and the q/k/score path run in native fp32 matmuls. The
value path (P@v', out-projection) tolerates tf32-class operand
rounding (validated by simulation at ~1e-2 max-rel-err vs the 2e-2
budget), so those matmuls run as single-pass float32r.

Per-core dataflow (residual transposed: x^T [1024, 512] in SBUF):
  P1: q^T, k^T fp32 projections in [do,s] layout; v in [s,do] layout
      with v' = v*reaches fused into the PSUM->SBUF copy (ACT engine).
  P2 per head: scores[q,k] (fp32) -> row-max (DVE reduce, negated) ->
      E = exp(s-m) with row-sum Z from the same ACT op (accum_out) ->
      P = (E*(-c/Z)[q])*diagmask in one scalar_tensor_tensor ->
      P transposed 128x128-blockwise via PE transpose-mode ->
      M2T[dk,q] = sum_k v'[k,dk]*PT[k,q] as f32r.
  P3: x += (OV@x)*c + ow-proj(concatT); OV = ow@vw folded on the host
      (fp32 matmul); the ow-proj runs f32r.

Engine notes: per-matmul fixed overhead (~200ns) makes one fp32
matmul (2 PE passes) cheaper than a hi/lo-split trio of f32r matmuls,
so projections are plain fp32. M2/OW use single f32r matmuls (1
pass). PSUM: psA(4)+psB(2)+psC(2) = 8 banks.
"""
import numpy as np

import concourse.tile as tile
from concourse import bacc, mybir
from concourse.bass_utils import run_bass_kernel_spmd

F32 = mybir.dt.float32
F32R = mybir.dt.float32r
BF16 = mybir.dt.bfloat16
AX = mybir.AxisListType
OP = mybir.AluOpType
AF = mybir.ActivationFunctionType

B, S, D, H, DK, NL = 8, 512, 1024, 16, 64, 4
QC = S // 128
KC = S // 128
DC = D // 128


TRACE = False        # test harness sets True for neuron-profile capture
LAST_RESULT = None   # BassKernelResults of the last kernel() call
_NC_CACHE = {}


def _build(n_layers=NL, n_cores=8):
    nc = bacc.Bacc("TRN2", target_bir_lowering=False, debug=False,
                   num_devices=n_cores)
    d_x0 = nc.dram_tensor("x0t", [D, S], F32, kind="ExternalInput").ap()
    dw = {}
    for nm in ["wq", "wk", "wv", "wov"]:
        dw[nm] = nc.dram_tensor(nm, [NL, D, D], F32, kind="ExternalInput").ap()
    dw["wo"] = nc.dram_tensor("wo", [NL, D, D], F32R, kind="ExternalInput").ap()
    d_cb = nc.dram_tensor("cb", [128, S], F32, kind="ExternalInput").ap()
    d_negc = nc.dram_tensor("negc", [128, QC], F32, kind="ExternalInput").ap()
    d_rr = nc.dram_tensor("rr", [128, KC], F32, kind="ExternalInput").ap()
    d_mask = nc.dram_tensor("maskq", [QC, 128, S], BF16, kind="ExternalInput").ap()
    d_id = nc.dram_tensor("ident", [128, 128], F32, kind="ExternalInput").ap()
    d_out = nc.dram_tensor("xt", [D, S], F32, kind="ExternalOutput").ap()

    with tile.TileContext(nc) as tc:
        _emit(nc, tc, n_layers, d_x0, dw,
              d_cb, d_negc, d_rr, d_mask, d_id, d_out)
    nc.compile()
    return nc


def _emit(nc, tc, n_layers, d_x0, dw, d_cb, d_negc, d_rr, d_mask, d_id, d_out):
    ctx_pools = []

    def pool(name, bufs, space="SBUF"):
        p = tc.tile_pool(name=name, bufs=bufs, space=space)
        ctx_pools.append(p)
        return p.__enter__()

    const = pool("const", 1)
    xpool = pool("x", 1)
    actp = pool("act", 1)
    wpool = pool("w", 1)          # 8 tags (per ki); reused across phases
    epool = pool("E", 3)
    ppool = pool("P", 5)
    ptpool = pool("PT", 5)
    small = pool("small", 4)
    tmp8 = pool("tmp8", 1)
    psA = pool("psA", 4, "PSUM")
    psB = pool("psB", 2, "PSUM")
    psC = pool("psC", 2, "PSUM")

    cb = const.tile([128, S], F32)
    nc.sync.dma_start(cb[:], d_cb)
    negc = const.tile([128, QC], F32)
    nc.sync.dma_start(negc[:], d_negc)
    rr = const.tile([128, KC], F32)
    nc.sync.dma_start(rr[:], d_rr)
    ident = const.tile([128, 128], F32)
    nc.sync.dma_start(ident[:], d_id)
    masks = []
    for t in range(QC):
        mt = const.tile([128, S], BF16, tag=f"mask{t}", name=f"mask{t}")
        nc.sync.dma_start(mt[:], d_mask[t])
        masks.append(mt)

    # residual x^T [D, S] fp32
    xt = []
    for c in range(DC):
        x = xpool.tile([128, S], F32, tag=f"x{c}", name=f"x{c}")
        nc.sync.dma_start(x[:], d_x0[c * 128:(c + 1) * 128, :])
        xt.append(x)

    for l in range(n_layers):
        def load_w(nm, dtype=F32):
            ws = []
            for ki in range(DC):
                w = wpool.tile([128, D], dtype, tag=f"w{ki}", name=f"{nm}{ki}_{l}")
                nc.sync.dma_start(w[:], dw[nm][l, ki * 128:(ki + 1) * 128, :])
                ws.append(w)
            return ws

        def proj(ws, outtag, psp):
            outs = []
            for c in range(DC):
                p = psp.tile([128, S], F32, tag=psp is psA and "psA" or "psC",
                             name=f"pp{outtag}{c}_{l}")
                sl = slice(c * 128, (c + 1) * 128)
                for ki in range(DC):
                    nc.tensor.matmul(p[:], ws[ki][:, sl], xt[ki][:],
                                     start=(ki == 0), stop=(ki == DC - 1),
                                     skip_group_check=True)
                outs.append(p)
            return outs

        wq = load_w("wq")
        qt = []
        for c, p in enumerate(proj(wq, "qt", psA)):
            o = actp.tile([128, S], F32, tag=f"qt{c}", name=f"qt{c}_{l}")
            nc.vector.tensor_copy(o[:], p[:])
            qt.append(o)
        wk = load_w("wk")
        kt = []
        for c, p in enumerate(proj(wk, "kt", psA)):
            o = actp.tile([128, S], F32, tag=f"kt{c}", name=f"kt{c}_{l}")
            nc.vector.tensor_copy(o[:], p[:])
            kt.append(o)

        # v in [s, dv] layout, scaled by reaches; f32r tiles for M2 lhsT
        wv = load_w("wv")
        vp = []
        for sc in range(KC):
            vtile = actp.tile([128, D], F32R, tag=f"vp{sc}", name=f"vp{sc}_{l}")
            ssl = slice(sc * 128, (sc + 1) * 128)
            for half in range(2):
                hsl = slice(half * 512, (half + 1) * 512)
                p = psA.tile([128, S], F32, tag="psA", name=f"pv{sc}{half}_{l}")
                for ki in range(DC):
                    nc.tensor.matmul(p[:], xt[ki][:, ssl], wv[ki][:, hsl],
                                     start=(ki == 0), stop=(ki == DC - 1),
                                     skip_group_check=True)
                nc.scalar.activation(vtile[:, hsl], p[:], AF.Copy,
                                     scale=rr[:, sc:sc + 1])
            vp.append(vtile)

        # OV projection (v-term folded with out-proj), fp32; t1 = (OV@x)*cb
        wov = load_w("wov")
        t1s = []
        for c, p in enumerate(proj(wov, "ov", psC)):
            t1 = tmp8.tile([128, S], F32, tag=f"t1{c}", name=f"t1{c}_{l}", bufs=1)
            nc.vector.tensor_tensor(t1[:], p[:], cb[:], op=OP.mult)
            t1s.append(t1)

        concatT = [actp.tile([128, S], F32R, tag=f"cc{c}", name=f"cc{c}_{l}")
                   for c in range(DC)]
        for h in range(H):
            hp = h // 2
            hb = (h % 2) * 64
            qsl = qt[hp][hb:hb + 64, :]
            ksl = kt[hp][hb:hb + 64, :]

            negm = small.tile([128, QC], F32, tag="negm", name=f"negm{h}_{l}")
            zst = small.tile([128, QC], F32, tag="zst", name=f"zst{h}_{l}")
            sc_t = small.tile([128, QC], F32, tag="scl", name=f"scl{h}_{l}")
            Ps = []
            for t in range(QC):
                ps = psA.tile([128, S], F32, tag="psA", name=f"sc{h}{t}_{l}")
                nc.tensor.matmul(ps[:], qsl[:, t * 128:(t + 1) * 128], ksl,
                                 start=True, stop=True)
                nc.vector.tensor_reduce(
                    negm[:, t:t + 1], ps[:], axis=AX.X, op=OP.max, negate=True)
                e = epool.tile([128, S], F32, tag="E", name=f"e{h}{t}_{l}")
                nc.scalar.activation(e[:], ps[:], AF.Exp,
                                     bias=negm[:, t:t + 1], scale=1.0,
                                     accum_out=zst[:, t:t + 1])
                nc.vector.reciprocal(sc_t[:, t:t + 1], zst[:, t:t + 1])
                nc.vector.tensor_tensor(
                    sc_t[:, t:t + 1], sc_t[:, t:t + 1], negc[:, t:t + 1],
                    op=OP.mult)
                p = ppool.tile([128, S], F32, tag="P", name=f"p{h}{t}_{l}")
                nc.vector.scalar_tensor_tensor(
                    p[:], e[:], sc_t[:, t:t + 1], masks[t][:],
                    op0=OP.mult, op1=OP.mult)
                Ps.append(p)

            PTs = []
            for kc in range(KC):
                tp = psB.tile([128, S], F32, tag="psB", name=f"tp{h}{kc}_{l}")
                for t in range(QC):
                    nc.tensor.matmul(
                        tp[:, t * 128:(t + 1) * 128],
                        Ps[t][:, kc * 128:(kc + 1) * 128], ident[:],
                        is_transpose=True, start=(t == 0), stop=(t == QC - 1),
                        skip_group_check=True)
                pt_sb = ptpool.tile([128, S], F32R, tag="PT", name=f"pt{h}{kc}_{l}")
                if kc % 2 == 0:
                    nc.vector.tensor_copy(pt_sb[:], tp[:])
                else:
                    nc.scalar.copy(pt_sb[:], tp[:])
                PTs.append(pt_sb)

            m2 = psC.tile([128, S], F32, tag="psC", name=f"m2{h}_{l}")
            for kc in range(KC):
                nc.tensor.matmul(
                    m2[0:64, :], vp[kc][:, h * 64:h * 64 + 64],
                    PTs[kc][:], start=(kc == 0), stop=(kc == KC - 1))
            nc.scalar.copy(concatT[hp][hb:hb + 64, :], m2[0:64, :])

        wo_t = load_w("wo", F32R)
        for c in range(DC):
            pow_ = psB.tile([128, S], F32, tag="psB", name=f"pow{c}_{l}")
            for ki in range(DC):
                nc.tensor.matmul(
                    pow_[:], wo_t[ki][:, c * 128:(c + 1) * 128], concatT[ki][:],
                    start=(ki == 0), stop=(ki == DC - 1))
            xt_new = tmp8.tile([128, S], F32, tag="xn", name=f"xn{c}_{l}", bufs=1)
            nc.vector.tensor_tensor(xt_new[:], pow_[:], t1s[c][:], op=OP.add)
            if l == n_layers - 1:
                xfin = tmp8.tile([128, S], F32, tag="xf", name=f"xf{c}_{l}",
                                 bufs=2)
                nc.vector.tensor_tensor(xfin[:], xt[c][:], xt_new[:], op=OP.add)
                nc.sync.dma_start(d_out[c * 128:(c + 1) * 128, :], xfin[:])
            else:
                nc.vector.tensor_tensor(xt[c][:], xt[c][:], xt_new[:], op=OP.add)

    for p in reversed(ctx_pools):
        p.__exit__(None, None, None)


# ---------------- host side ----------------

def _host_prep(src, reaches, emb_table, qw, kw, vw, ow):
    src = np.asarray(src)
    reaches = np.asarray(reaches, dtype=np.float32)
    emb_table = np.asarray(emb_table, dtype=np.float32)
    emb = emb_table[src]
    rs = reaches.sum(-1, keepdims=True)
    contrib = ((rs - reaches) / (rs + 1e-9) * (1.0 - reaches) * 100.0
               ).astype(np.float32)

    qw = np.asarray(qw, np.float32); kw = np.asarray(kw, np.float32)
    vw = np.asarray(vw, np.float32); ow = np.asarray(ow, np.float32)
    wq = np.ascontiguousarray(np.transpose(qw, (0, 2, 1)) * 0.125)
    wk = np.ascontiguousarray(np.transpose(kw, (0, 2, 1)))
    wv = np.ascontiguousarray(np.transpose(vw, (0, 2, 1)))
    wo = np.ascontiguousarray(np.transpose(ow, (0, 2, 1)))
    wov = np.stack([
        np.ascontiguousarray(
            (ow[l].astype(np.float64) @ vw[l].astype(np.float64)).T
        ).astype(np.float32)
        for l in range(NL)])

    import ml_dtypes
    maskq = np.ones((QC, 128, S), ml_dtypes.bfloat16)
    idx = np.arange(128)
    diagval = np.float32(1.0) - np.float32(0.999999)
    for t in range(QC):
        maskq[t, idx, t * 128 + idx] = ml_dtypes.bfloat16(diagval)
    ident = np.eye(128, dtype=np.float32)

    shared = dict(wq=wq, wk=wk, wv=wv, wov=wov, wo=wo, maskq=maskq,
                  ident=ident)
    in_maps = []
    for b in range(B):
        in_maps.append(dict(
            shared,
            x0t=np.ascontiguousarray(emb[b].T),
            cb=np.ascontiguousarray(
                np.broadcast_to(contrib[b][None, :], (128, S))),
            negc=np.ascontiguousarray(-contrib[b].reshape(QC, 128).T),
            rr=np.ascontiguousarray(reaches[b].reshape(KC, 128).T),
        ))
    return emb, in_maps


def kernel(src, reaches, emb_table, qw, kw, vw, ow):
    global LAST_RESULT
    if "nc" not in _NC_CACHE:
        _NC_CACHE["nc"] = _build(n_layers=NL, n_cores=B)
    nc = _NC_CACHE["nc"]
    emb, in_maps = _host_prep(src, reaches, emb_table, qw, kw, vw, ow)
    res = run_bass_kernel_spmd(nc, in_maps, core_ids=list(range(B)),
                               trace=TRACE)
    LAST_RESULT = res
    x = np.stack([r["xt"].T for r in res.results]).astype(np.float32)
    return emb, x


# revision 5
# speedup vs baseline: 1.4369x; 1.0880x over previous
"""TRN2 Bass kernel for the 4-layer encoder-with-reaches model
(nn_EncoderPreTre: B=8, S=512, D=1024, H=16 heads, NL=4 layers).

kernel(**inputs) takes the FULL inputs (src, reaches, emb_table,
qw/kw/vw/ow) and returns the full output tuple (emb, x) matching
reference.reference(). Distribution: data-parallel over the batch —
core b computes batch element b end to end (B == 8 == n_cores); the
embedding-row gather and per-batch contrib vectors are the host-side
sharding step.

Numerics: the model amplifies score-path rounding hugely (logits reach
~4e6 by layer 3; contrib scaling grows x ~40x per layer), so the
residual x and the q/k/score path run in native fp32 matmuls. The
value path (P@v', out-projection) tolerates tf32-class operand
rounding (validated by simulation at ~1e-2 max-rel-err vs the 2e-2
budget), so those matmuls run as single-pass float32r.

Per-core dataflow (residual transposed: x^T [1024, 512] in SBUF):
  P1: q^T, k^T fp32 projections in [do,s] layout; v in [s,do] layout
      with v' = v*reaches fused into the PSUM->SBUF copy (ACT engine).
  P2 per head: scores[q,k] (fp32) -> row-max (DVE reduce, negated) ->
      E = exp(s-m) with row-sum Z from the same ACT op (accum_out) ->
      P = (E*(-c/Z)[q])*diagmask in one scalar_tensor_tensor ->
      P transposed 128x128-blockwise via PE transpose-mode ->
      M2T[dk,q] = sum_k v'[k,dk]*PT[k,q] as f32r.
  P3: x += (OV@x)*c + ow-proj(concatT); OV = ow@vw folded on the host
      (fp32 matmul); the ow-proj runs f32r.

Engine notes: per-matmul fixed overhead (~200ns) makes one fp32
matmul (2 PE passes) cheaper than a hi/lo-split trio of f32r matmuls,
so projections are plain fp32. M2/OW use single f32r matmuls (1
pass). PSUM: psA(4)+psB(2)+psC(2) = 8 banks.
"""
import numpy as np

import concourse.tile as tile
from concourse import bacc, mybir
from concourse.bass_utils import run_bass_kernel_spmd

F32 = mybir.dt.float32
F32R = mybir.dt.float32r
BF16 = mybir.dt.bfloat16
FP16 = mybir.dt.float16
AX = mybir.AxisListType
OP = mybir.AluOpType
AF = mybir.ActivationFunctionType

B, S, D, H, DK, NL = 8, 512, 1024, 16, 64, 4
QC = S // 128
KC = S // 128
DC = D // 128


TRACE = False        # test harness sets True for neuron-profile capture
LAST_RESULT = None   # BassKernelResults of the last kernel() call
_NC_CACHE = {}


def _build(n_layers=NL, n_cores=8):
    nc = bacc.Bacc("TRN2", target_bir_lowering=False, debug=False,
                   num_devices=n_cores)
    d_x0 = nc.dram_tensor("x0t", [D, S], F32, kind="ExternalInput").ap()
    dw = {}
    for nm in ["wq", "wk", "wv", "wov"]:
        dw[nm] = nc.dram_tensor(nm, [NL, D, D], F32, kind="ExternalInput").ap()
    # f32r views of the q/k weights for the low-precision early layers
    for nm in ["wqr", "wkr"]:
        dw[nm] = nc.dram_tensor(nm, [2, D, D], F32R, kind="ExternalInput").ap()
    dw["wo"] = nc.dram_tensor("wo", [NL, D, D], F32R, kind="ExternalInput").ap()
    dw["wo16"] = nc.dram_tensor("wo16", [NL, D, D], FP16, kind="ExternalInput").ap()
    d_cb = nc.dram_tensor("cb", [128, S], F32, kind="ExternalInput").ap()
    d_negc = nc.dram_tensor("negc", [128, QC], F32, kind="ExternalInput").ap()
    d_rr = nc.dram_tensor("rr", [128, KC], F32, kind="ExternalInput").ap()
    d_mask = nc.dram_tensor("maskq", [QC, 128, S], BF16, kind="ExternalInput").ap()
    d_id = nc.dram_tensor("ident", [128, 128], F32, kind="ExternalInput").ap()
    d_out = nc.dram_tensor("xt", [D, S], F32, kind="ExternalOutput").ap()

    with tile.TileContext(nc) as tc:
        _emit(nc, tc, n_layers, d_x0, dw,
              d_cb, d_negc, d_rr, d_mask, d_id, d_out)
    nc.compile()
    return nc


def _emit(nc, tc, n_layers, d_x0, dw, d_cb, d_negc, d_rr, d_mask, d_id, d_out):
    ctx_pools = []

    def pool(name, bufs, space="SBUF"):
        p = tc.tile_pool(name=name, bufs=bufs, space=space)
        ctx_pools.append(p)
        return p.__enter__()

    const = pool("const", 1)
    xpool = pool("x", 1)
    actp = pool("act", 1)
    wpool = pool("w", 1)          # 8 tags (per ki); reused across phases
    epool = pool("E", 3)
    ppool = pool("P", 5)
    ptpool = pool("PT", 5)
    small = pool("small", 4)
    tmp8 = pool("tmp8", 1)
    psA = pool("psA", 4, "PSUM")
    psB = pool("psB", 2, "PSUM")
    psC = pool("psC", 2, "PSUM")

    cb = const.tile([128, S], F32)
    nc.sync.dma_start(cb[:], d_cb)
    negc = const.tile([128, QC], F32)
    nc.sync.dma_start(negc[:], d_negc)
    rr = const.tile([128, KC], F32)
    nc.sync.dma_start(rr[:], d_rr)
    ident = const.tile([128, 128], F32)
    nc.sync.dma_start(ident[:], d_id)
    masks = []
    for t in range(QC):
        mt = const.tile([128, S], BF16, tag=f"mask{t}", name=f"mask{t}")
        nc.sync.dma_start(mt[:], d_mask[t])
        masks.append(mt)

    # residual x^T [D, S] fp32
    xt = []
    for c in range(DC):
        x = xpool.tile([128, S], F32, tag=f"x{c}", name=f"x{c}")
        nc.sync.dma_start(x[:], d_x0[c * 128:(c + 1) * 128, :])
        xt.append(x)

    for l in range(n_layers):
        def load_w(nm, dtype=F32):
            ws = []
            for ki in range(DC):
                w = wpool.tile([128, D], dtype, tag=f"w{ki}", name=f"{nm}{ki}_{l}")
                nc.sync.dma_start(w[:], dw[nm][l, ki * 128:(ki + 1) * 128, :])
                ws.append(w)
            return ws

        def proj(ws, outtag, psp):
            outs = []
            for c in range(DC):
                p = psp.tile([128, S], F32, tag=psp is psA and "psA" or "psC",
                             name=f"pp{outtag}{c}_{l}")
                sl = slice(c * 128, (c + 1) * 128)
                for ki in range(DC):
                    nc.tensor.matmul(p[:], ws[ki][:, sl], xt[ki][:],
                                     start=(ki == 0), stop=(ki == DC - 1),
                                     skip_group_check=True)
                outs.append(p)
            return outs

        wq = load_w("wq")
        qt = []
        for c, p in enumerate(proj(wq, "qt", psA)):
            o = actp.tile([128, S], F32, tag=f"qt{c}", name=f"qt{c}_{l}")
            nc.vector.tensor_copy(o[:], p[:])
            qt.append(o)
        wk = load_w("wk")
        kt = []
        for c, p in enumerate(proj(wk, "kt", psA)):
            o = actp.tile([128, S], F32, tag=f"kt{c}", name=f"kt{c}_{l}")
            nc.vector.tensor_copy(o[:], p[:])
            kt.append(o)

        # v in [s, dv] layout, scaled by reaches; f32r tiles for M2 lhsT
        wv = load_w("wv")
        vp = []
        for sc in range(KC):
            vtile = actp.tile([128, D], vdt, tag=f"vp{sc}", name=f"vp{sc}_{l}")
            ssl = slice(sc * 128, (sc + 1) * 128)
            for half in range(2):
                hsl = slice(half * 512, (half + 1) * 512)
                p = psA.tile([128, S], F32, tag="psA", name=f"pv{sc}{half}_{l}")
                for ki in range(DC):
                    nc.tensor.matmul(p[:], xt[ki][:, ssl], wv[ki][:, hsl],
                                     start=(ki == 0), stop=(ki == DC - 1),
                                     skip_group_check=True)
                nc.scalar.activation(vtile[:, hsl], p[:], AF.Copy,
                                     scale=rr[:, sc:sc + 1])
            vp.append(vtile)

        # OV projection (v-term folded with out-proj), fp32; t1 = (OV@x)*cb
        wov = load_w("wov")
        t1s = []
        for c, p in enumerate(proj(wov, "ov", psC)):
            t1 = tmp8.tile([128, S], F32, tag=f"t1{c}", name=f"t1{c}_{l}", bufs=1)
            nc.vector.tensor_tensor(t1[:], p[:], cb[:], op=OP.mult)
            t1s.append(t1)

        concatT = [actp.tile([128, S], vdt, tag=f"cc{c}", name=f"cc{c}_{l}")
                   for c in range(DC)]
        for h in range(H):
            hp = h // 2
            hb = (h % 2) * 64
            qsl = qt[hp][hb:hb + 64, :]
            ksl = kt[hp][hb:hb + 64, :]

            negm = small.tile([128, QC], F32, tag="negm", name=f"negm{h}_{l}")
            zst = small.tile([128, QC], F32, tag="zst", name=f"zst{h}_{l}")
            sc_t = small.tile([128, QC], F32, tag="scl", name=f"scl{h}_{l}")
            Ps = []
            for t in range(QC):
                ps = psA.tile([128, S], F32, tag="psA", name=f"sc{h}{t}_{l}")
                nc.tensor.matmul(ps[:], qsl[:, t * 128:(t + 1) * 128], ksl,
                                 start=True, stop=True)
                nc.vector.tensor_reduce(
                    negm[:, t:t + 1], ps[:], axis=AX.X, op=OP.max, negate=True)
                e = epool.tile([128, S], F32, tag="E", name=f"e{h}{t}_{l}")
                nc.scalar.activation(e[:], ps[:], AF.Exp,
                                     bias=negm[:, t:t + 1], scale=1.0,
                                     accum_out=zst[:, t:t + 1])
                nc.vector.reciprocal(sc_t[:, t:t + 1], zst[:, t:t + 1])
                nc.vector.tensor_tensor(
                    sc_t[:, t:t + 1], sc_t[:, t:t + 1], negc[:, t:t + 1],
                    op=OP.mult)
                p = ppool.tile([128, S], F32, tag="P", name=f"p{h}{t}_{l}")
                nc.vector.scalar_tensor_tensor(
                    p[:], e[:], sc_t[:, t:t + 1], masks[t][:],
                    op0=OP.mult, op1=OP.mult)
                Ps.append(p)

            PTs = []
            for kc in range(KC):
                tp = psB.tile([128, S], F32, tag="psB", name=f"tp{h}{kc}_{l}")
                for t in range(QC):
                    nc.tensor.matmul(
                        tp[:, t * 128:(t + 1) * 128],
                        Ps[t][:, kc * 128:(kc + 1) * 128], ident[:],
                        is_transpose=True, start=(t == 0), stop=(t == QC - 1),
                        skip_group_check=True)
                pt_sb = ptpool.tile([128, S], vdt, tag="PT", name=f"pt{h}{kc}_{l}")
                if kc % 2 == 0:
                    nc.vector.tensor_copy(pt_sb[:], tp[:])
                else:
                    nc.scalar.copy(pt_sb[:], tp[:])
                PTs.append(pt_sb)

            m2 = psC.tile([128, S], F32, tag="psC", name=f"m2{h}_{l}")
            for kc in range(KC):
                nc.tensor.matmul(
                    m2[0:64, :], vp[kc][:, h * 64:h * 64 + 64],
                    PTs[kc][:], start=(kc == 0), stop=(kc == KC - 1))
            nc.scalar.copy(concatT[hp][hb:hb + 64, :], m2[0:64, :])

        wo_t = load_w("wo16" if l < 3 else "wo", vdt)
        for c in range(DC):
            pow_ = psB.tile([128, S], F32, tag="psB", name=f"pow{c}_{l}")
            for ki in range(DC):
                nc.tensor.matmul(
                    pow_[:], wo_t[ki][:, c * 128:(c + 1) * 128], concatT[ki][:],
                    start=(ki == 0), stop=(ki == DC - 1))
            xt_new = tmp8.tile([128, S], F32, tag="xn", name=f"xn{c}_{l}", bufs=1)
            nc.vector.tensor_tensor(xt_new[:], pow_[:], t1s[c][:], op=OP.add)
            if l == n_layers - 1:
                xfin = tmp8.tile([128, S], F32, tag="xf", name=f"xf{c}_{l}",
                                 bufs=2)
                nc.vector.tensor_tensor(xfin[:], xt[c][:], xt_new[:], op=OP.add)
                nc.sync.dma_start(d_out[c * 128:(c + 1) * 128, :], xfin[:])
            else:
                nc.vector.tensor_tensor(xt[c][:], xt[c][:], xt_new[:], op=OP.add)

    for p in reversed(ctx_pools):
        p.__exit__(None, None, None)


# ---------------- host side ----------------

def _host_prep(src, reaches, emb_table, qw, kw, vw, ow):
    src = np.asarray(src)
    reaches = np.asarray(reaches, dtype=np.float32)
    emb_table = np.asarray(emb_table, dtype=np.float32)
    emb = emb_table[src]
    rs = reaches.sum(-1, keepdims=True)
    contrib = ((rs - reaches) / (rs + 1e-9) * (1.0 - reaches) * 100.0
               ).astype(np.float32)

    qw = np.asarray(qw, np.float32); kw = np.asarray(kw, np.float32)
    vw = np.asarray(vw, np.float32); ow = np.asarray(ow, np.float32)
    wq = np.ascontiguousarray(np.transpose(qw, (0, 2, 1)) * 0.125)
    wk = np.ascontiguousarray(np.transpose(kw, (0, 2, 1)))
    wv = np.ascontiguousarray(np.transpose(vw, (0, 2, 1)))
    wo = np.ascontiguousarray(np.transpose(ow, (0, 2, 1)))
    wov = np.stack([
        np.ascontiguousarray(
            (ow[l].astype(np.float64) @ vw[l].astype(np.float64)).T
        ).astype(np.float32)
        for l in range(NL)])

    import ml_dtypes
    maskq = np.ones((QC, 128, S), ml_dtypes.bfloat16)
    idx = np.arange(128)
    diagval = np.float32(1.0) - np.float32(0.999999)
    for t in range(QC):
        maskq[t, idx, t * 128 + idx] = ml_dtypes.bfloat16(diagval)
    ident = np.eye(128, dtype=np.float32)

    import ml_dtypes
    wo16 = wo.astype(ml_dtypes.float16 if hasattr(ml_dtypes, 'float16') else np.float16)
    shared = dict(wq=wq, wk=wk, wv=wv, wov=wov, wo=wo, wo16=wo16, maskq=maskq,
                  ident=ident)
    in_maps = []
    for b in range(B):
        in_maps.append(dict(
            shared,
            x0t=np.ascontiguousarray(emb[b].T),
            cb=np.ascontiguousarray(
                np.broadcast_to(contrib[b][None, :], (128, S))),
            negc=np.ascontiguousarray(-contrib[b].reshape(QC, 128).T),
            rr=np.ascontiguousarray(reaches[b].reshape(KC, 128).T),
        ))
    return emb, in_maps


def kernel(src, reaches, emb_table, qw, kw, vw, ow):
    global LAST_RESULT
    if "nc" not in _NC_CACHE:
        _NC_CACHE["nc"] = _build(n_layers=NL, n_cores=B)
    nc = _NC_CACHE["nc"]
    emb, in_maps = _host_prep(src, reaches, emb_table, qw, kw, vw, ow)
    res = run_bass_kernel_spmd(nc, in_maps, core_ids=list(range(B)),
                               trace=TRACE)
    LAST_RESULT = res
    x = np.stack([r["xt"].T for r in res.results]).astype(np.float32)
    return emb, x


# revision 6
# speedup vs baseline: 1.5209x; 1.0584x over previous
"""TRN2 Bass kernel for the 4-layer encoder-with-reaches model
(nn_EncoderPreTre: B=8, S=512, D=1024, H=16 heads, NL=4 layers).

kernel(**inputs) takes the FULL inputs (src, reaches, emb_table,
qw/kw/vw/ow) and returns the full output tuple (emb, x) matching
reference.reference(). Distribution: data-parallel over the batch —
core b computes batch element b end to end (B == 8 == n_cores); the
embedding-row gather and per-batch contrib vectors are the host-side
sharding step.

Numerics: the model amplifies score-path rounding hugely (logits reach
~4e6 by layer 3; contrib scaling grows x ~40x per layer), so the
residual x and the q/k/score path run in native fp32 matmuls. The
value path (P@v', out-projection) tolerates tf32-class operand
rounding (validated by simulation at ~1e-2 max-rel-err vs the 2e-2
budget), so those matmuls run as single-pass float32r.

Per-core dataflow (residual transposed: x^T [1024, 512] in SBUF):
  P1: q^T, k^T fp32 projections in [do,s] layout; v in [s,do] layout
      with v' = v*reaches fused into the PSUM->SBUF copy (ACT engine).
  P2 per head: scores[q,k] (fp32) -> row-max (DVE reduce, negated) ->
      E = exp(s-m) with row-sum Z from the same ACT op (accum_out) ->
      P = (E*(-c/Z)[q])*diagmask in one scalar_tensor_tensor ->
      P transposed 128x128-blockwise via PE transpose-mode ->
      M2T[dk,q] = sum_k v'[k,dk]*PT[k,q] as f32r.
  P3: x += (OV@x)*c + ow-proj(concatT); OV = ow@vw folded on the host
      (fp32 matmul); the ow-proj runs f32r.

Engine notes: per-matmul fixed overhead (~200ns) makes one fp32
matmul (2 PE passes) cheaper than a hi/lo-split trio of f32r matmuls,
so projections are plain fp32. M2/OW use single f32r matmuls (1
pass). PSUM: psA(4)+psB(2)+psC(2) = 8 banks.
"""
import numpy as np

import concourse.tile as tile
from concourse import bacc, mybir
from concourse.bass_utils import run_bass_kernel_spmd

F32 = mybir.dt.float32
F32R = mybir.dt.float32r
BF16 = mybir.dt.bfloat16
FP16 = mybir.dt.float16
AX = mybir.AxisListType
OP = mybir.AluOpType
AF = mybir.ActivationFunctionType

B, S, D, H, DK, NL = 8, 512, 1024, 16, 64, 4
RNS = 1.000244140625   # 1 + 2^-12: half-ulp pre-scale so RTZ fp16 converts round-to-nearest
QC = S // 128
KC = S // 128
DC = D // 128


TRACE = False        # test harness sets True for neuron-profile capture
LAST_RESULT = None   # BassKernelResults of the last kernel() call
_NC_CACHE = {}


def _build(n_layers=NL, n_cores=8):
    nc = bacc.Bacc("TRN2", target_bir_lowering=False, debug=False,
                   num_devices=n_cores)
    d_x0 = nc.dram_tensor("x0t", [D, S], F32, kind="ExternalInput").ap()
    dw = {}
    for nm in ["wq", "wk", "wv", "wov"]:
        dw[nm] = nc.dram_tensor(nm, [NL, D, D], F32, kind="ExternalInput").ap()
    # f32r views of the q/k weights for the low-precision early layers
    for nm in ["wqr", "wkr"]:
        dw[nm] = nc.dram_tensor(nm, [2, D, D], F32R, kind="ExternalInput").ap()
    dw["wo"] = nc.dram_tensor("wo", [NL, D, D], F32R, kind="ExternalInput").ap()
    dw["wo16"] = nc.dram_tensor("wo16", [NL, D, D], FP16, kind="ExternalInput").ap()
    dw["wvr"] = nc.dram_tensor("wvr", [1, D, D], F32R, kind="ExternalInput").ap()
    dw["wovr"] = nc.dram_tensor("wovr", [1, D, D], F32R, kind="ExternalInput").ap()
    d_cb = nc.dram_tensor("cb", [128, S], F32, kind="ExternalInput").ap()
    d_negc = nc.dram_tensor("negc", [128, QC], F32, kind="ExternalInput").ap()
    d_rr = nc.dram_tensor("rr", [128, KC], F32, kind="ExternalInput").ap()
    d_rrn = nc.dram_tensor("rrn", [128, KC], F32, kind="ExternalInput").ap()
    d_mask = nc.dram_tensor("maskq", [QC, 128, S], BF16, kind="ExternalInput").ap()
    d_id = nc.dram_tensor("ident", [128, 128], F32, kind="ExternalInput").ap()
    d_out = nc.dram_tensor("xt", [D, S], F32, kind="ExternalOutput").ap()

    with tile.TileContext(nc) as tc:
        _emit(nc, tc, n_layers, d_x0, dw,
              d_cb, d_negc, d_rr, d_rrn, d_mask, d_id, d_out)
    nc.compile()
    return nc


def _emit(nc, tc, n_layers, d_x0, dw, d_cb, d_negc, d_rr, d_rrn, d_mask, d_id, d_out):
    ctx_pools = []

    def pool(name, bufs, space="SBUF"):
        p = tc.tile_pool(name=name, bufs=bufs, space=space)
        ctx_pools.append(p)
        return p.__enter__()

    const = pool("const", 1)
    xpool = pool("x", 1)
    actp = pool("act", 1)
    wpool = pool("w", 1)          # 8 tags (per ki); reused across phases
    epool = pool("E", 3)
    ppool = pool("P", 5)
    ptpool = pool("PT", 5)
    small = pool("small", 4)
    tmp8 = pool("tmp8", 1)
    psA = pool("psA", 4, "PSUM")
    psB = pool("psB", 2, "PSUM")
    psC = pool("psC", 2, "PSUM")

    cb = const.tile([128, S], F32)
    nc.sync.dma_start(cb[:], d_cb)
    negc = const.tile([128, QC], F32)
    nc.sync.dma_start(negc[:], d_negc)
    rr = const.tile([128, KC], F32)
    nc.sync.dma_start(rr[:], d_rr)
    rrn = const.tile([128, KC], F32, tag="rrn", name="rrn")
    nc.sync.dma_start(rrn[:], d_rrn)
    ident = const.tile([128, 128], F32)
    nc.sync.dma_start(ident[:], d_id)
    masks = []
    for t in range(QC):
        mt = const.tile([128, S], BF16, tag=f"mask{t}", name=f"mask{t}")
        nc.sync.dma_start(mt[:], d_mask[t])
        masks.append(mt)

    # residual x^T [D, S] fp32
    xt = []
    for c in range(DC):
        x = xpool.tile([128, S], F32, tag=f"x{c}", name=f"x{c}")
        nc.sync.dma_start(x[:], d_x0[c * 128:(c + 1) * 128, :])
        xt.append(x)

    for l in range(n_layers):
        def load_w(nm, dtype=F32, idx=None):
            li = l if idx is None else idx
            ws = []
            for ki in range(DC):
                w = wpool.tile([128, D], dtype, tag=f"w{ki}", name=f"{nm}{ki}_{l}")
                nc.sync.dma_start(w[:], dw[nm][li, ki * 128:(ki + 1) * 128, :])
                ws.append(w)
            return ws

        def proj(ws, outtag, psp):
            outs = []
            for c in range(DC):
                p = psp.tile([128, S], F32, tag=psp is psA and "psA" or "psC",
                             name=f"pp{outtag}{c}_{l}")
                sl = slice(c * 128, (c + 1) * 128)
                for ki in range(DC):
                    nc.tensor.matmul(p[:], ws[ki][:, sl], xt[ki][:],
                                     start=(ki == 0), stop=(ki == DC - 1),
                                     skip_group_check=True)
                outs.append(p)
            return outs

        wq = load_w("wq")
        qt = []
        for c, p in enumerate(proj(wq, "qt", psA)):
            o = actp.tile([128, S], F32, tag=f"qt{c}", name=f"qt{c}_{l}")
            nc.vector.tensor_copy(o[:], p[:])
            qt.append(o)
        wk = load_w("wk")
        kt = []
        for c, p in enumerate(proj(wk, "kt", psA)):
            o = actp.tile([128, S], F32, tag=f"kt{c}", name=f"kt{c}_{l}")
            nc.vector.tensor_copy(o[:], p[:])
            kt.append(o)

        # v in [s, dv] layout, scaled by reaches; f32r tiles for M2 lhsT
        wv = load_w("wvr", F32R, idx=0) if last else load_w("wv")
        vx = xr if last else xt
        vscale = rr if last else rrn
        vp = []
        for sc in range(KC):
            vtile = actp.tile([128, D], vdt, tag=f"vp{sc}", name=f"vp{sc}_{l}")
            ssl = slice(sc * 128, (sc + 1) * 128)
            for half in range(2):
                hsl = slice(half * 512, (half + 1) * 512)
                p = psA.tile([128, S], F32, tag="psA", name=f"pv{sc}{half}_{l}")
                for ki in range(DC):
                    nc.tensor.matmul(p[:], vx[ki][:, ssl], wv[ki][:, hsl],
                                     start=(ki == 0), stop=(ki == DC - 1),
                                     skip_group_check=True)
                nc.scalar.activation(vtile[:, hsl], p[:], AF.Copy,
                                     scale=vscale[:, sc:sc + 1])
            vp.append(vtile)

        # OV projection (v-term folded with out-proj), fp32; t1 = (OV@x)*cb
        wov = load_w("wovr", F32R, idx=0) if last else load_w("wov")
        t1s = []
        for c, p in enumerate(proj(wov, "ov", psC, xr if last else xt)):
            t1 = tmp8.tile([128, S], F32, tag=f"t1{c}", name=f"t1{c}_{l}", bufs=1)
            nc.vector.tensor_tensor(t1[:], p[:], cb[:], op=OP.mult)
            t1s.append(t1)

        concatT = [actp.tile([128, S], vdt, tag=f"cc{c}", name=f"cc{c}_{l}")
                   for c in range(DC)]
        for h in range(H):
            hp = h // 2
            hb = (h % 2) * 64
            qsl = qt[hp][hb:hb + 64, :]
            ksl = kt[hp][hb:hb + 64, :]

            negm = small.tile([128, QC], F32, tag="negm", name=f"negm{h}_{l}")
            zst = small.tile([128, QC], F32, tag="zst", name=f"zst{h}_{l}")
            sc_t = small.tile([128, QC], F32, tag="scl", name=f"scl{h}_{l}")
            Ps = []
            for t in range(QC):
                ps = psA.tile([128, S], F32, tag="psA", name=f"sc{h}{t}_{l}")
                nc.tensor.matmul(ps[:], qsl[:, t * 128:(t + 1) * 128], ksl,
                                 start=True, stop=True)
                nc.vector.tensor_reduce(
                    negm[:, t:t + 1], ps[:], axis=AX.X, op=OP.max, negate=True)
                e = epool.tile([128, S], F32, tag="E", name=f"e{h}{t}_{l}")
                nc.scalar.activation(e[:], ps[:], AF.Exp,
                                     bias=negm[:, t:t + 1], scale=1.0,
                                     accum_out=zst[:, t:t + 1])
                nc.vector.reciprocal(sc_t[:, t:t + 1], zst[:, t:t + 1])
                nc.vector.tensor_tensor(
                    sc_t[:, t:t + 1], sc_t[:, t:t + 1], negc[:, t:t + 1],
                    op=OP.mult)
                p = ppool.tile([128, S], F32, tag="P", name=f"p{h}{t}_{l}")
                nc.vector.scalar_tensor_tensor(
                    p[:], e[:], sc_t[:, t:t + 1], masks[t][:],
                    op0=OP.mult, op1=OP.mult)
                Ps.append(p)

            PTs = []
            for kc in range(KC):
                tp = psB.tile([128, S], F32, tag="psB", name=f"tp{h}{kc}_{l}")
                for t in range(QC):
                    nc.tensor.matmul(
                        tp[:, t * 128:(t + 1) * 128],
                        Ps[t][:, kc * 128:(kc + 1) * 128], ident[:],
                        is_transpose=True, start=(t == 0), stop=(t == QC - 1),
                        skip_group_check=True)
                pt_sb = ptpool.tile([128, S], vdt, tag="PT", name=f"pt{h}{kc}_{l}")
                if kc % 2 == 0:
                    if l < 3:
                        nc.vector.tensor_scalar(pt_sb[:], tp[:], RNS, None,
                                                op0=OP.mult)
                    else:
                        nc.vector.tensor_copy(pt_sb[:], tp[:])
                else:
                    if l < 3:
                        nc.scalar.activation(pt_sb[:], tp[:], AF.Copy, scale=RNS)
                    else:
                        nc.scalar.copy(pt_sb[:], tp[:])
                PTs.append(pt_sb)

            m2 = psC.tile([128, S], F32, tag="psC", name=f"m2{h}_{l}")
            for kc in range(KC):
                nc.tensor.matmul(
                    m2[0:64, :], vp[kc][:, h * 64:h * 64 + 64],
                    PTs[kc][:], start=(kc == 0), stop=(kc == KC - 1))
            if l < 3:
                nc.scalar.activation(concatT[hp][hb:hb + 64, :], m2[0:64, :],
                                     AF.Copy, scale=RNS)
            else:
                nc.scalar.copy(concatT[hp][hb:hb + 64, :], m2[0:64, :])

        wo_t = load_w("wo16" if l < 3 else "wo", vdt)
        for c in range(DC):
            pow_ = psB.tile([128, S], F32, tag="psB", name=f"pow{c}_{l}")
            for ki in range(DC):
                nc.tensor.matmul(
                    pow_[:], wo_t[ki][:, c * 128:(c + 1) * 128], concatT[ki][:],
                    start=(ki == 0), stop=(ki == DC - 1))
            xt_new = tmp8.tile([128, S], F32, tag="xn", name=f"xn{c}_{l}", bufs=1)
            nc.vector.tensor_tensor(xt_new[:], pow_[:], t1s[c][:], op=OP.add)
            if l == n_layers - 1:
                xfin = tmp8.tile([128, S], F32, tag="xf", name=f"xf{c}_{l}",
                                 bufs=2)
                nc.vector.tensor_tensor(xfin[:], xt[c][:], xt_new[:], op=OP.add)
                nc.sync.dma_start(d_out[c * 128:(c + 1) * 128, :], xfin[:])
            else:
                nc.vector.tensor_tensor(xt[c][:], xt[c][:], xt_new[:], op=OP.add)

    for p in reversed(ctx_pools):
        p.__exit__(None, None, None)


# ---------------- host side ----------------

def _host_prep(src, reaches, emb_table, qw, kw, vw, ow):
    src = np.asarray(src)
    reaches = np.asarray(reaches, dtype=np.float32)
    emb_table = np.asarray(emb_table, dtype=np.float32)
    emb = emb_table[src]
    rs = reaches.sum(-1, keepdims=True)
    contrib = ((rs - reaches) / (rs + 1e-9) * (1.0 - reaches) * 100.0
               ).astype(np.float32)

    qw = np.asarray(qw, np.float32); kw = np.asarray(kw, np.float32)
    vw = np.asarray(vw, np.float32); ow = np.asarray(ow, np.float32)
    wq = np.ascontiguousarray(np.transpose(qw, (0, 2, 1)) * 0.125)
    wk = np.ascontiguousarray(np.transpose(kw, (0, 2, 1)))
    wv = np.ascontiguousarray(np.transpose(vw, (0, 2, 1)))
    wo = np.ascontiguousarray(np.transpose(ow, (0, 2, 1)))
    wov = np.stack([
        np.ascontiguousarray(
            (ow[l].astype(np.float64) @ vw[l].astype(np.float64)).T
        ).astype(np.float32)
        for l in range(NL)])

    import ml_dtypes
    maskq = np.ones((QC, 128, S), ml_dtypes.bfloat16)
    idx = np.arange(128)
    diagval = np.float32(1.0) - np.float32(0.999999)
    for t in range(QC):
        maskq[t, idx, t * 128 + idx] = ml_dtypes.bfloat16(diagval)
    ident = np.eye(128, dtype=np.float32)

    import ml_dtypes
    wo16 = wo.astype(ml_dtypes.float16 if hasattr(ml_dtypes, 'float16') else np.float16)
    shared = dict(wq=wq, wk=wk, wv=wv, wov=wov, wo=wo, wo16=wo16, maskq=maskq,
                  ident=ident)
    in_maps = []
    for b in range(B):
        in_maps.append(dict(
            shared,
            x0t=np.ascontiguousarray(emb[b].T),
            cb=np.ascontiguousarray(
                np.broadcast_to(contrib[b][None, :], (128, S))),
            negc=np.ascontiguousarray(-contrib[b].reshape(QC, 128).T),
            rr=np.ascontiguousarray(reaches[b].reshape(KC, 128).T),
            rrn=np.ascontiguousarray(
                reaches[b].reshape(KC, 128).T * np.float32(RNS)),
        ))
    return emb, in_maps


def kernel(src, reaches, emb_table, qw, kw, vw, ow):
    global LAST_RESULT
    if "nc" not in _NC_CACHE:
        _NC_CACHE["nc"] = _build(n_layers=NL, n_cores=B)
    nc = _NC_CACHE["nc"]
    emb, in_maps = _host_prep(src, reaches, emb_table, qw, kw, vw, ow)
    res = run_bass_kernel_spmd(nc, in_maps, core_ids=list(range(B)),
                               trace=TRACE)
    LAST_RESULT = res
    x = np.stack([r["xt"].T for r in res.results]).astype(np.float32)
    return emb, x
